# revision 1
# baseline (speedup 1.0000x reference)
"""Chamfer distance (squared-L2, mean of both directional min-means) on 8
Trainium2 NeuronCores.

Sharding: B=16 batches of N=M=4096 3-D points, data-parallel, 2 batches per
core.  Each core computes row-mins of the full 4096x4096 squared-distance
matrix in both directions (D and D^T) and returns per-row minima; the host
sums and averages (both directional means share the denominator B*N).

Device kernel, per batch and direction ("mix" mode):
  * One K=32 stacked bf16 matmul per 128-row chunk emits negated distance
    tiles -D[n, m] = -(|q_n|^2 + |k_m|^2 - 2 q_n.k_m) into fp32 PSUM.  The
    fp32 coordinates are split host-side into three bf16 components
    (hi/mid/lo) and all cross products above ~2^-32 relative magnitude are
    kept, so the bf16 TensorE (1 cycle/column; native fp32 matmul is 4x
    slower) reproduces fp32-accuracy distances.  |q|^2 / |k|^2 enter the
    same matmul as four-way bf16 splits against ones-rows, and the global
    negation is folded into the lhsT operand.
  * 7 of 8 chunks: ScalarE cast-copies each [128, 2048] PSUM half-stripe to
    fp16 SBUF with scale=-1 (restoring +D), then the DVE pair-mins the two
    halves (fp16 2x mode) and min-reduces to a row-min column.
  * Every 8th chunk: the DVE max-reduces -D fp32 straight from PSUM
    (reduce max is faster than min on TRN2 - the 8-comparator max8 path),
    keeping ScalarE and VectorE pipelines balanced.
  * PSUM is double-buffered ([128, 2048] x 2) so TensorE fills one
    half-stripe while the reducers drain the other.

The host negates the fp32 maxima, clamps at zero (identical to the
reference's maximum(d, 0) because clamping commutes with min) and averages
in f64.

Measured on 8 axon-attached TRN2 cores: ~0.58 ms hardware time per
dispatch (repeat-loop slope method), relative error vs the fp32 jax
reference ~5e-6.
"""

import sys
from contextlib import ExitStack

import numpy as np

sys.path.insert(0, "/opt/trn_rl_repo")

import ml_dtypes

import concourse.bass as bass
import concourse.tile as tile
from concourse import bacc, mybir
from concourse.bass_utils import run_bass_kernel_spmd

B, N, M = 16, 4096, 4096
NCORES = 8
BPC = B // NCORES          # batches per core
NDIR = 2 * BPC             # matmul directions per core (2 per batch)
K = 32                     # stacked contraction rows
NCHUNK = N // 128          # 32 output-row chunks per direction
# MODE "f32max": matmul emits -D, DVE max-reduces fp32 straight from PSUM
#   (the DVE max8 path is measurably faster than min) — exact fp32, no ACT.
# MODE "f16": ACT casts PSUM->fp16, DVE 16-bit min ops.
# MODE "f32min": plain fp32 min-reduce from PSUM.
# MODE "mix": negated stacks; most chunks ACT-cast to fp16 with scale=-1
#   (restoring +D) then fast DVE fp16 min; every HYB-th chunk max-reduces
#   -D fp32 straight from PSUM — balances ACT and DVE.
MODE = "mix"
NEGATED = MODE in ("f32max", "mix")
REDUCE_F16 = MODE in ("f16", "mix")
DECOUPLED = False          # independent per-half cast->reduce pipelines
PACKED = False             # 4 concurrent row-group matmuls (tile_position)
NG = NDIR * NCHUNK         # total chunks per core
# Every HYB-th chunk bypasses ACT and reduces fp32 straight from PSUM on the
# DVE — balances the ACT (cast) and DVE (min) pipelines.
HYB = 8
F32_CHUNKS = [g for g in range(NG) if g % HYB == HYB - 1] if REDUCE_F16 else \
    list(range(NG))
RM_COLS = 2 * NG
RM32_COLS = 2 * len(F32_CHUNKS) if REDUCE_F16 else 2
HALF = 2048                # half-stripe width (4 PSUM banks)
BF16 = ml_dtypes.bfloat16
BIG = float(np.finfo(np.float32).max)


# ----------------------------------------------------------------- host prep

def _splitn(x, n):
    """x (fp32/fp64) -> n bf16 arrays p_i with sum(p_i) = x + O(2^-(8n) x)."""
    parts = []
    r = x
    for _ in range(n):
        p = r.astype(BF16)
        parts.append(p)
        r = r - p.astype(x.dtype)
    return parts


def _stacks(z):
    """z: [N, 3] fp32 points -> (lhsT_stack [K, N] bf16, rhs_stack [K, N] bf16).

    Row pairing (lhsT row k multiplies rhs row k, summed over k): the 3-way
    bf16 split of each coordinate (h/m/l) keeps all cross products except
    l.l (2^-32 relative); |z|^2 enters as a 4-way bf16 split against a
    ones-row on the opposite side.
      k 0-8  : (-2 h1).(h2|m2|l2)    k 9-17 : (-2 m1).(h2|m2|l2)
      k 18-23: (-2 l1).(h2|m2)       k 24-27: sq1 parts . 1
      k 28-31: 1 . sq2 parts
    """
    zt = np.ascontiguousarray(z.T.astype(np.float32))          # [3, N]
    h, m, l = _splitn(zt, 3)
    sq = (z.astype(np.float64) ** 2).sum(axis=-1)              # [N]
    sqp = _splitn(sq, 4)
    npts = z.shape[0]

    lhs = np.empty((K, npts), dtype=BF16)
    h2 = (-2.0 * h.astype(np.float32)).astype(BF16)            # exact (power of 2)
    m2 = (-2.0 * m.astype(np.float32)).astype(BF16)
    l2 = (-2.0 * l.astype(np.float32)).astype(BF16)
    for i, a in enumerate((h2, h2, h2, m2, m2, m2, l2, l2)):
        lhs[3 * i: 3 * i + 3] = a
    for i in range(4):
        lhs[24 + i] = sqp[i]
    lhs[28:32] = np.ones((4, npts), dtype=BF16)

    rhs = np.empty((K, npts), dtype=BF16)
    for i, a in enumerate((h, m, l, h, m, l, h, m)):
        rhs[3 * i: 3 * i + 3] = a
    rhs[24:28] = np.ones((4, npts), dtype=BF16)
    for i in range(4):
        rhs[28 + i] = sqp[i]
    if NEGATED:
        lhs = -lhs                # PSUM accumulates -D; reduce becomes max
    return lhs, rhs


# -------------------------------------------------------------- device build

def _build_nc(repeat=1):
    """repeat > 1 builds a timing variant: the full compute loop re-executes
    `repeat` times inside one NEFF (same data, idempotent) so per-pass
    hardware time can be extracted from the wall-clock slope."""
    nc = bacc.Bacc("TRN2", target_bir_lowering=False, debug=False)
    krep = 128 if PACKED else K
    lhs_d = nc.dram_tensor("lhs", [NDIR, krep, N], mybir.dt.bfloat16,
                           kind="ExternalInput")
    rhs_d = nc.dram_tensor("rhs", [NDIR, krep, N], mybir.dt.bfloat16,
                           kind="ExternalInput")
    rm_d = nc.dram_tensor(
        "rowmins", [128, RM_COLS],
        mybir.dt.float16 if REDUCE_F16 else mybir.dt.float32,
        kind="ExternalOutput")
    rm32_d = nc.dram_tensor("rowmins32", [128, max(RM32_COLS, 2)],
                            mybir.dt.float32, kind="ExternalOutput")
    lhs_ap, rhs_ap, rm_ap = lhs_d.ap(), rhs_d.ap(), rm_d.ap()

    with tile.TileContext(nc) as tc, ExitStack() as ctx:
        stacks = ctx.enter_context(tc.tile_pool(name="stacks", bufs=1))
        psum = ctx.enter_context(
            tc.tile_pool(name="psum", bufs=2, space="PSUM"))
        qpool = ctx.enter_context(tc.tile_pool(name="qcast", bufs=10))
        rmpool = ctx.enter_context(tc.tile_pool(name="rm", bufs=1))

        lhs_t, rhs_t = [], []
        for s in range(NDIR):
            lt = stacks.tile([krep, N], mybir.dt.bfloat16, tag=f"lhs{s}")
            nc.sync.dma_start(lt[:], lhs_ap[s])
            rt = stacks.tile([krep, N], mybir.dt.bfloat16, tag=f"rhs{s}")
            nc.sync.dma_start(rt[:], rhs_ap[s])
            lhs_t.append(lt)
            rhs_t.append(rt)

        rm_dt = mybir.dt.float16 if REDUCE_F16 else mybir.dt.float32
        rm = rmpool.tile([128, RM_COLS], rm_dt)
        rm32 = rmpool.tile([128, max(RM32_COLS, 2)], mybir.dt.float32)
        nc.gpsimd.memset(rm[:], 60000.0)   # fp16-safe "+inf" for unused cols
        nc.gpsimd.memset(rm32[:], 0.0)
        f32_col = {g: i for i, g in enumerate(F32_CHUNKS)}

        def body():
            for s in range(NDIR):
                lt = lhs_t[s]
                rt = rhs_t[s ^ 1]      # query side s pairs with the other tensor
                for c in range(NCHUNK):
                    pa = psum.tile([128, HALF], mybir.dt.float32, tag="ps")
                    pb = psum.tile([128, HALF], mybir.dt.float32, tag="ps")
                    for h, pt in ((0, pa), (1, pb)):
                        for j in range(4):
                            if PACKED:
                                # row-group j runs its matmul concurrently
                                # with the other groups (same weights data,
                                # replicated per 32-partition group).
                                rg = 32 * j
                                nc.tensor.matmul(
                                    pt[:, j * 512:(j + 1) * 512],
                                    lt[rg:rg + K, c * 128:(c + 1) * 128],
                                    rt[rg:rg + K,
                                       h * HALF + j * 512:
                                       h * HALF + (j + 1) * 512],
                                    tile_position=(rg, 0))
                            else:
                                nc.tensor.matmul(
                                    pt[:, j * 512:(j + 1) * 512],
                                    lt[:, c * 128:(c + 1) * 128],
                                    rt[:, h * HALF + j * 512:
                                       h * HALF + (j + 1) * 512])
                    g = s * NCHUNK + c
                    if REDUCE_F16 and g not in f32_col:
                        # ACT cast-copies fp32 PSUM -> fp16 SBUF; DVE then
                        # min-reduces in its 2x 16-bit mode.
                        if DECOUPLED:
                            for h, ph in ((0, pa), (1, pb)):
                                qh = qpool.tile([128, HALF],
                                                mybir.dt.float16, tag=f"q{h}")
                                (nc.scalar.mul(qh[:], ph[:], -1.0)
                                 if NEGATED else
                                 nc.scalar.copy(qh[:], ph[:]))
                                nc.vector.tensor_reduce(
                                    rm[:, 2 * g + h:2 * g + h + 1], qh[:],
                                    axis=mybir.AxisListType.X,
                                    op=mybir.AluOpType.min)
                        else:
                            qa = qpool.tile([128, HALF], mybir.dt.float16,
                                            tag="q0")
                            (nc.scalar.mul(qa[:], pa[:], -1.0)
                             if NEGATED else nc.scalar.copy(qa[:], pa[:]))
                            qb = qpool.tile([128, HALF], mybir.dt.float16,
                                            tag="q1")
                            (nc.scalar.mul(qb[:], pb[:], -1.0)
                             if NEGATED else nc.scalar.copy(qb[:], pb[:]))
                            t1 = qpool.tile([128, HALF], mybir.dt.float16,
                                            tag="t1")
                            nc.vector.tensor_tensor(
                                t1[:], qa[:], qb[:], mybir.AluOpType.min)
                            nc.vector.tensor_reduce(
                                rm[:, 2 * g:2 * g + 1], t1[:],
                                axis=mybir.AxisListType.X,
                                op=mybir.AluOpType.min)
                    else:
                        col = 2 * (f32_col[g] if REDUCE_F16 else g)
                        dst = rm32 if REDUCE_F16 else rm
                        red_op = (mybir.AluOpType.max if NEGATED
                                  else mybir.AluOpType.min)
                        nc.vector.tensor_reduce(
                            dst[:, col:col + 1], pa[:],
                            axis=mybir.AxisListType.X, op=red_op)
                        nc.vector.tensor_reduce(
                            dst[:, col + 1:col + 2], pb[:],
                            axis=mybir.AxisListType.X, op=red_op)

        if repeat > 1:
            with tc.For_i(0, repeat, 1):
                body()
        else:
            body()
        nc.sync.dma_start(rm_ap, rm[:])
        nc.sync.dma_start(rm32_d.ap(), rm32[:])
    nc.compile()
    return nc


_CACHE: dict = {}


def _get_nc():
    if "nc" not in _CACHE:
        _CACHE["nc"] = _build_nc()
    return _CACHE["nc"]


# --------------------------------------------------------------------- entry

def make_in_maps(xyz1, xyz2):
    in_maps = []
    krep = 128 if PACKED else K
    for core in range(NCORES):
        lhs = np.empty((NDIR, krep, N), dtype=BF16)
        rhs = np.empty((NDIR, krep, N), dtype=BF16)
        for bl in range(BPC):
            b = core * BPC + bl
            for t, z in ((0, xyz1[b]), (1, xyz2[b])):
                ls, rs = _stacks(np.asarray(z))
                if PACKED:
                    ls = np.tile(ls, (128 // K, 1))
                    rs = np.tile(rs, (128 // K, 1))
                lhs[bl * 2 + t], rhs[bl * 2 + t] = ls, rs
        in_maps.append({"lhs": lhs, "rhs": rhs})
    return in_maps


def combine(results):
    f16_mask = np.array([g not in set(F32_CHUNKS) for g in range(NG)])
    total = 0.0
    for r in results:
        rm = r["rowmins"].astype(np.float64)
        if NEGATED and not REDUCE_F16:
            rm = -rm                               # stored -D maxima
        rm = rm.reshape(128, NG, 2).min(axis=-1)   # pair half-stripe mins
        if REDUCE_F16:
            total += np.maximum(rm[:, f16_mask], 0.0).sum()
            rm32 = r["rowmins32"].astype(np.float64)
            if NEGATED:
                rm32 = -rm32                       # stored -D maxima
            rm32 = rm32.reshape(128, -1, 2).min(axis=-1)
            total += np.maximum(rm32, 0.0).sum()
        else:
            total += np.maximum(rm, 0.0).sum()
    return np.float32(total / (B * N))


def kernel(xyz1, xyz2, **_):
    in_maps = make_in_maps(xyz1, xyz2)
    try:
        res = run_bass_kernel_spmd(_get_nc(), in_maps,
                                   core_ids=list(range(NCORES)))
    except Exception:                      # transient axon/PJRT hiccup
        _CACHE.clear()
        res = run_bass_kernel_spmd(_get_nc(), in_maps,
                                   core_ids=list(range(NCORES)))
    return combine(res.results)



# revision 3
# speedup vs baseline: 2.3806x; 2.3806x over previous
"""Chamfer distance (squared-L2, mean of both directional min-means) on 8
Trainium2 NeuronCores — symmetric single-matmul variant.

Sharding: B=16 batches of N=M=4096 3-D points, data-parallel, 2 batches per
core.  Unlike the two-pass baseline (which ran one matmul per direction,
2x 4096x4096 per batch), each batch's distance matrix is computed ONCE:
  * dist1 (min over columns for each row)   = row-max of -D
  * dist2 (min over rows for each column)   = col-max of -D
so TensorE work and the number of PSUM elements the reducer pipeline must
touch are both halved.

Device kernel, per batch:
  * One K=32 stacked bf16 matmul per 128-row chunk emits negated distance
    tiles -D[n, m] into fp32 PSUM (two [128, 2048] half-stripes, the same
    hi/mid/lo bf16 splitting as the baseline: fp32-accurate distances).
  * ScalarE cast-copies each half-stripe to fp16 halves of one [128, 4096]
    SBUF tile q.
  * VectorE (all fp16 2x-mode tensor_tensor):
      - col-max accumulate: acc = max(acc, q)          (1 op, 4096 wide)
      - row-max tree: 2048 <- 1024 <- 512 <- reduce    (4 ops)
    The row maxima (one scalar per row) land in rm[:, b*32+c].
  * After the 32 chunks: acc [128, 4096] holds per-partition column maxima.
    TensorE transposes it 128x128-tile-wise (identity matmul) into PSUM and
    the DVE reduces the transposed stripes to per-column maxima cm.

The host negates rm/cm (restoring +dist mins), clamps at zero (identical to
the reference's maximum(d, 0): clamping commutes with min) and averages in
f64.
"""

import sys
from contextlib import ExitStack

import numpy as np

sys.path.insert(0, "/opt/trn_rl_repo")

import ml_dtypes

import concourse.bass as bass
import concourse.tile as tile
from concourse import bacc, mybir
from concourse.bass_utils import run_bass_kernel_spmd

B, N, M = 16, 4096, 4096
NCORES = 8
BPC = B // NCORES          # batches per core
K = 32                     # stacked contraction rows
NCHUNK = N // 128          # 32 output-row chunks per batch
HALF = 2048                # half-stripe width (4 PSUM banks)
BF16 = ml_dtypes.bfloat16
NEG_BIG = -60000.0         # fp16-safe "-inf" for max-accumulators


# ----------------------------------------------------------------- host prep

def _splitn(x, n):
    """x (fp32/fp64) -> n bf16 arrays p_i with sum(p_i) = x + O(2^-(8n) x)."""
    parts = []
    r = x
    for _ in range(n):
        p = r.astype(BF16)
        parts.append(p)
        r = r - p.astype(x.dtype)
    return parts


def _stacks(z):
    """z: [N, 3] fp32 points -> (lhsT_stack [K, N] bf16, rhs_stack [K, N] bf16).

    Row pairing (lhsT row k multiplies rhs row k, summed over k): the 3-way
    bf16 split of each coordinate (h/m/l) keeps all cross products except
    l.l (2^-32 relative); |z|^2 enters as a 4-way bf16 split against a
    ones-row on the opposite side.  lhsT is globally negated so PSUM
    accumulates -D.
      k 0-8  : (-2 h1).(h2|m2|l2)    k 9-17 : (-2 m1).(h2|m2|l2)
      k 18-23: (-2 l1).(h2|m2)       k 24-27: sq1 parts . 1
      k 28-31: 1 . sq2 parts
    """
    zt = np.ascontiguousarray(z.T.astype(np.float32))          # [3, N]
    h, m, l = _splitn(zt, 3)
    sq = (z.astype(np.float64) ** 2).sum(axis=-1)              # [N]
    sqp = _splitn(sq, 4)
    npts = z.shape[0]

    lhs = np.empty((K, npts), dtype=BF16)
    h2 = (-2.0 * h.astype(np.float32)).astype(BF16)            # exact (power of 2)
    m2 = (-2.0 * m.astype(np.float32)).astype(BF16)
    l2 = (-2.0 * l.astype(np.float32)).astype(BF16)
    for i, a in enumerate((h2, h2, h2, m2, m2, m2, l2, l2)):
        lhs[3 * i: 3 * i + 3] = a
    for i in range(4):
        lhs[24 + i] = sqp[i]
    lhs[28:32] = np.ones((4, npts), dtype=BF16)

    rhs = np.empty((K, npts), dtype=BF16)
    for i, a in enumerate((h, m, l, h, m, l, h, m)):
        rhs[3 * i: 3 * i + 3] = a
    rhs[24:28] = np.ones((4, npts), dtype=BF16)
    for i in range(4):
        rhs[28 + i] = sqp[i]
    return -lhs, rhs           # negated: PSUM accumulates -D, reduce is max


# -------------------------------------------------------------- device build

def _build_nc(repeat=1):
    """repeat > 1 builds a timing variant: the full compute loop re-executes
    `repeat` times inside one NEFF (same data, idempotent: max-accumulators
    are absorbing) so per-pass hardware time can be extracted from the
    wall-clock slope."""
    nc = bacc.Bacc("TRN2", target_bir_lowering=False, debug=False)
    lhs_d = nc.dram_tensor("lhs", [BPC, K, N], mybir.dt.bfloat16,
                           kind="ExternalInput")
    rhs_d = nc.dram_tensor("rhs", [BPC, K, M], mybir.dt.bfloat16,
                           kind="ExternalInput")
    eye_d = nc.dram_tensor("eye", [128, 128], mybir.dt.float16,
                           kind="ExternalInput")
    rm_d = nc.dram_tensor("rowmax", [128, BPC * NCHUNK], mybir.dt.float32,
                          kind="ExternalOutput")
    cm_d = nc.dram_tensor("colmax", [128, BPC * (M // 128)], mybir.dt.float32,
                          kind="ExternalOutput")
    lhs_ap, rhs_ap = lhs_d.ap(), rhs_d.ap()

    with tile.TileContext(nc) as tc, ExitStack() as ctx:
        stacks = ctx.enter_context(tc.tile_pool(name="stacks", bufs=1))
        psum = ctx.enter_context(
            tc.tile_pool(name="psum", bufs=2, space="PSUM"))
        qpool = ctx.enter_context(tc.tile_pool(name="qcast", bufs=4))
        tpool = ctx.enter_context(tc.tile_pool(name="tree", bufs=2))
        apool = ctx.enter_context(tc.tile_pool(name="accs", bufs=1))
        rpool = ctx.enter_context(tc.tile_pool(name="res", bufs=1))

        lhs_t, rhs_t = [], []
        for b in range(BPC):
            lt = stacks.tile([K, N], mybir.dt.bfloat16, tag=f"lhs{b}")
            nc.sync.dma_start(lt[:], lhs_ap[b])
            rt = stacks.tile([K, M], mybir.dt.bfloat16, tag=f"rhs{b}")
            nc.sync.dma_start(rt[:], rhs_ap[b])
            lhs_t.append(lt)
            rhs_t.append(rt)
        eye_t = stacks.tile([128, 128], mybir.dt.float16, tag="eye")
        nc.sync.dma_start(eye_t[:], eye_d.ap())

        rm = rpool.tile([128, BPC * NCHUNK], mybir.dt.float32, tag="rm")
        cm = rpool.tile([128, BPC * (M // 128)], mybir.dt.float32, tag="cm")
        accs = []
        for b in range(BPC):
            acc = apool.tile([128, M], mybir.dt.float16, tag=f"acc{b}")
            nc.gpsimd.memset(acc[:], NEG_BIG)
            accs.append(acc)

        def body():
            for b in range(BPC):
                acc, lt, rt = accs[b], lhs_t[b], rhs_t[b]
                for c in range(NCHUNK):
                    pa = psum.tile([128, HALF], mybir.dt.float32, tag="ps")
                    pb = psum.tile([128, HALF], mybir.dt.float32, tag="ps")
                    for h, pt in ((0, pa), (1, pb)):
                        for j in range(4):
                            nc.tensor.matmul(
                                pt[:, j * 512:(j + 1) * 512],
                                lt[:, c * 128:(c + 1) * 128],
                                rt[:, h * HALF + j * 512:
                                   h * HALF + (j + 1) * 512])
                    q = qpool.tile([128, N], mybir.dt.float16, tag="q")
                    nc.scalar.copy(q[:, 0:HALF], pa[:])
                    nc.scalar.copy(q[:, HALF:N], pb[:])
                    # col-max accumulate (elementwise, in place, fp16 2x)
                    nc.vector.tensor_tensor(
                        acc[:], acc[:], q[:], mybir.AluOpType.max)
                    # row-max tree (fp16 2x tensor_tensor halving)
                    t1 = tpool.tile([128, 2048], mybir.dt.float16, tag="t1")
                    nc.vector.tensor_tensor(
                        t1[:], q[:, 0:HALF], q[:, HALF:N],
                        mybir.AluOpType.max)
                    t2 = tpool.tile([128, 1024], mybir.dt.float16, tag="t2")
                    nc.vector.tensor_tensor(
                        t2[:], t1[:, 0:1024], t1[:, 1024:2048],
                        mybir.AluOpType.max)
                    t3 = tpool.tile([128, 512], mybir.dt.float16, tag="t3")
                    nc.vector.tensor_tensor(
                        t3[:], t2[:, 0:512], t2[:, 512:1024],
                        mybir.AluOpType.max)
                    g = b * NCHUNK + c
                    nc.vector.tensor_reduce(
                        rm[:, g:g + 1], t3[:],
                        axis=mybir.AxisListType.X, op=mybir.AluOpType.max)
                # finalize col-max: transpose acc tile-wise, reduce over rows
                for s in range(2):
                    pt = psum.tile([128, 16, 128], mybir.dt.float16, tag="ps")
                    for j in range(16):
                        nc.tensor.transpose(
                            pt[:, j],
                            acc[:, (s * 16 + j) * 128:(s * 16 + j + 1) * 128],
                            eye_t[:])
                    col = (b * 2 + s) * 16
                    nc.vector.tensor_reduce(
                        cm[:, col:col + 16], pt[:],
                        axis=mybir.AxisListType.X, op=mybir.AluOpType.max)

        if repeat > 1:
            with tc.For_i(0, repeat, 1):
                body()
        else:
            body()
        nc.sync.dma_start(rm_d.ap(), rm[:])
        nc.sync.dma_start(cm_d.ap(), cm[:])
    nc.compile()
    return nc


_CACHE: dict = {}


def _get_nc():
    if "nc" not in _CACHE:
        _CACHE["nc"] = _build_nc()
    return _CACHE["nc"]


# --------------------------------------------------------------------- entry

def make_in_maps(xyz1, xyz2):
    eye = np.eye(128, dtype=np.float16)
    in_maps = []
    for core in range(NCORES):
        lhs = np.empty((BPC, K, N), dtype=BF16)
        rhs = np.empty((BPC, K, M), dtype=BF16)
        for bl in range(BPC):
            b = core * BPC + bl
            ls, _ = _stacks(np.asarray(xyz1[b]))
            _, rs = _stacks(np.asarray(xyz2[b]))
            lhs[bl], rhs[bl] = ls, rs
        in_maps.append({"lhs": lhs, "rhs": rhs, "eye": eye})
    return in_maps


def combine(results):
    total = 0.0
    for r in results:
        rm = -r["rowmax"].astype(np.float64)   # [128, 64] -> dist1 mins
        cm = -r["colmax"].astype(np.float64)   # [128, 64] -> dist2 mins
        total += np.maximum(rm, 0.0).sum() + np.maximum(cm, 0.0).sum()
    return np.float32(total / (B * N))


def kernel(xyz1, xyz2, **_):
    in_maps = make_in_maps(xyz1, xyz2)
    try:
        res = run_bass_kernel_spmd(_get_nc(), in_maps,
                                   core_ids=list(range(NCORES)))
    except Exception:                      # transient axon/PJRT hiccup
        _CACHE.clear()
        res = run_bass_kernel_spmd(_get_nc(), in_maps,
                                   core_ids=list(range(NCORES)))
    return combine(res.results)
